# revision 9
# baseline (speedup 1.0000x reference)
"""Trainium2 Bass kernel for CoarseningRegularizerMx loss.

loss[i] = mean_{j != i, Mx[j]==Mx[i]} ||z_i - z_j||_2

Block-diagonal scheme: only same-label pairs contribute, and label
blocks are small (~32 rows for B=8192, 256 labels). HOST-side we
bin-pack whole label blocks into 128-row chunks (FFD), so every row's
positives live INSIDE its own chunk. Each chunk then needs only its own
128 columns — no windows, no overlaps: the distance work is exactly one
[128 x 128] tile per chunk per K-tile.

Device pass per core (nch chunks of 128 rows, nch ~ 9):
  - 4 plain-fp8 matmuls per chunk (K=512 as 4 tiles of 128) computing
    +Q_i.Q_j into PSUM. No DoubleRow: 128-col fp8 weights trigger the
    compiler's Fast Weight Load (4 elem/cycle), so LDWEIGHTS (~27ns)
    hides fully under the rhs streams; DoubleRow's 256-col weight loads
    (~213ns each) were the baseline's bottleneck.
  - 1 augmented matmul (K=5) adding -(qnorm_i + qnorm_j + 2)/2 where
    qnorm is the EXACT norm of the quantized row (host hi/lo fp8 split).
  - ONE Activation per rep: dist = sqrt(-2 * psum) over [128, nch*128]
    (scale=-2 folded into the activation). Every d2 is
    ||Q_i - Q_j||^2 + 2 >= ~1 > 0 including the diagonal, so sqrt is
    NaN-safe with no masking first.
  - DVE: one wide mask multiply (host-precomputed 0/1 bf16 mask zeroing
    the diagonal, cross-label pairs and padding), two pairwise
    fold-adds, a small TensorReduce, one multiply by 1/n_select.
Output is produced in packed order and scattered back on the host.
"""

import numpy as np
import ml_dtypes

import concourse.bass as bass  # noqa: F401
from concourse import bacc
import concourse.mybir as mybir
import concourse.tile as tile
from concourse.bass_utils import run_bass_kernel_spmd

BF16 = ml_dtypes.bfloat16
FP8 = mybir.dt.np(mybir.dt.float8e4)          # ml_dtypes.float8_e4m3

B, D = 8192, 512
NCORES = 8
P = 128               # partitions / chunk rows
KT = D // P           # 4 K-tiles of 128
AUGK = 5              # augmented-contraction rows
SQC = 511.0           # qnorm centering: d = (SQC - qnorm)/2

_cache = {}


def _build_bass(repeat: int, nch: int, dr: bool = True) -> bacc.Bacc:
    nc = bacc.Bacc(None, target_bir_lowering=False)
    dt = mybir.dt

    if dr:
        # DoubleRow layout [p, s, t, c, r]: k = t*256 + s*128 + p
        zt = nc.dram_tensor("zt", [P, 2, KT // 2, nch, P], dt.float8e4,
                            kind="ExternalInput")
    else:
        zt = nc.dram_tensor("zt", [P, KT, nch, P], dt.float8e4, kind="ExternalInput")
    aug_l = nc.dram_tensor("aug_l", [AUGK, nch, P], dt.float8e4,
                           kind="ExternalInput")
    aug_r = nc.dram_tensor("aug_r", [AUGK, nch, P], dt.float8e4,
                           kind="ExternalInput")
    mask = nc.dram_tensor("mask", [P, nch, P], dt.bfloat16, kind="ExternalInput")
    invn = nc.dram_tensor("invn", [nch, P], dt.float32, kind="ExternalInput")
    rep_tag = nc.dram_tensor("rep_tag", [1, max(repeat, 1)], dt.float32,
                             kind="ExternalInput")
    loss = nc.dram_tensor("loss", [nch, P], dt.float32, kind="ExternalOutput")

    with tile.TileContext(nc) as tc:
        with (
            tc.tile_pool(name="singles", bufs=1) as singles,
            tc.tile_pool(name="dist_pool", bufs=2) as dist_pool,
            tc.tile_pool(name="fold_pool", bufs=2) as fold_pool,
            tc.tile_pool(name="red_pool", bufs=2) as red_pool,
            tc.tile_pool(name="psum", bufs=2, space="PSUM") as psum_pool,
        ):
            # --- one-time loads -------------------------------------------------
            zt_sb = singles.tile(list(zt.shape), dt.float8e4)
            nc.sync.dma_start(out=zt_sb, in_=zt[...])
            aug_l_sb = singles.tile([AUGK, nch, P], dt.float8e4)
            nc.sync.dma_start(out=aug_l_sb, in_=aug_l[:, :, :])
            aug_r_sb = singles.tile([AUGK, nch, P], dt.float8e4)
            nc.sync.dma_start(out=aug_r_sb, in_=aug_r[:, :, :])
            mask_sb = singles.tile([P, nch, P], dt.bfloat16)
            nc.sync.dma_start(out=mask_sb, in_=mask[:, :, :])
            invn_sb = singles.tile([P, nch], dt.float32)
            nc.sync.dma_start(out=invn_sb, in_=invn.rearrange("m p -> p m"))
            rep_tag_sb = singles.tile([1, max(repeat, 1)], dt.float32)
            nc.sync.dma_start(out=rep_tag_sb, in_=rep_tag[:, :])

            loss_sb = singles.tile([P, nch], dt.float32)

            # Make the DVE observe the mask/invn DMAs once, so the per-rep
            # ops don't each need sync waits on those DMA queues.
            dve_warm = singles.tile([P, 1], dt.float32)
            nc.vector.tensor_tensor(
                dve_warm, mask_sb[:, 0, :1], invn_sb[:, :1],
                op=mybir.AluOpType.mult,
            )

            # --- main loop ------------------------------------------------------
            for _rep in range(repeat):
                psum = psum_pool.tile([P, nch, P], dt.float32,
                                      name="ps", tag="ps")
                for c in range(nch):
                    if dr:
                        for t in range(KT // 2):
                            nc.tensor.matmul(
                                psum[:, c, :],
                                lhsT=zt_sb[:, :, t, c, :],
                                rhs=zt_sb[:, :, t, c, :],
                                start=(t == 0),
                                stop=False,
                                perf_mode=mybir.MatmulPerfMode.DoubleRow,
                            )
                    else:
                        for t in range(KT):
                            nc.tensor.matmul(
                                psum[:, c, :],
                                lhsT=zt_sb[:, t, c, :],
                                rhs=zt_sb[:, t, c, :],
                                start=(t == 0),
                                stop=False,
                            )
                    nc.tensor.matmul(
                        psum[:, c, :],
                        lhsT=aug_l_sb[:, c, :],
                        rhs=aug_r_sb[:, c, :],
                        start=False,
                        stop=True,
                    )
                dist = dist_pool.tile([P, nch, P], dt.bfloat16,
                                      name="dist", tag="dist")
                nc.scalar.activation(
                    out=dist, in_=psum,
                    func=mybir.ActivationFunctionType.Sqrt,
                    scale=-2.0,
                )
                md = fold_pool.tile([P, nch, P], dt.bfloat16,
                                    name="md", tag="md")
                nc.vector.tensor_tensor(
                    md, mask_sb, dist, op=mybir.AluOpType.mult,
                )
                f1 = fold_pool.tile([P, nch, P // 2], dt.bfloat16,
                                    name="f1", tag="f1")
                nc.vector.tensor_tensor(
                    f1, md[:, :, 0:P // 2], md[:, :, P // 2:P],
                    op=mybir.AluOpType.add,
                )
                f2 = fold_pool.tile([P, nch, P // 4], dt.bfloat16,
                                    name="f2", tag="f2")
                nc.vector.tensor_tensor(
                    f2, f1[:, :, 0:P // 4], f1[:, :, P // 4:P // 2],
                    op=mybir.AluOpType.add,
                )
                red = red_pool.tile([P, nch], dt.float32, name="red", tag="red")
                nc.vector.reduce_sum(red, f2, axis=mybir.AxisListType.X)
                nc.vector.tensor_tensor(
                    loss_sb, red, invn_sb, op=mybir.AluOpType.mult,
                )

            nc.sync.dma_start(out=loss.rearrange("m p -> p m"), in_=loss_sb)

    return nc


def _fp8_hilo(x: np.ndarray) -> tuple[np.ndarray, np.ndarray]:
    hi = x.astype(FP8)
    lo = (x - hi.astype(np.float32)).astype(FP8)
    return hi, lo


def _pack(Mx: np.ndarray):
    """FFD bin-pack whole label blocks into 128-row chunks.

    Returns slots [n_bins_padded, P] of row indices (-1 = padding)."""
    counts = np.bincount(Mx, minlength=1)
    order = np.argsort(Mx, kind="stable")
    starts = np.zeros(len(counts) + 1, dtype=np.int64)
    np.cumsum(counts, out=starts[1:])
    sizes = sorted(((int(c), int(l)) for l, c in enumerate(counts) if c > 0),
                   reverse=True)
    if sizes and sizes[0][0] > P:
        raise NotImplementedError(
            f"label block of {sizes[0][0]} rows exceeds chunk size {P}")
    bins: list[list[int]] = []
    room: list[int] = []
    for c, l in sizes:
        for b in range(len(bins)):
            if room[b] >= c:
                bins[b].append(l)
                room[b] -= c
                break
        else:
            bins.append([l])
            room.append(P - c)
    while len(bins) % NCORES:
        bins.append([])
    slots = np.full((len(bins), P), -1, dtype=np.int64)
    for b, labels in enumerate(bins):
        pos = 0
        for l in labels:
            n = int(counts[l])
            slots[b, pos:pos + n] = order[starts[l]:starts[l] + n]
            pos += n
    return slots


def _prepare_inputs(z: np.ndarray, Mx: np.ndarray, slots: np.ndarray,
                    repeat: int = 1, dr: bool = True):
    z = np.ascontiguousarray(z, dtype=np.float32)
    Mx = np.asarray(Mx).astype(np.int64)
    nch = slots.shape[0] // NCORES

    zq = z.astype(FP8)
    zqf = zq.astype(np.float32)
    qnorm = np.einsum("ij,ij->i", zqf, zqf, dtype=np.float32)
    # aug adds -(qnorm_i + qnorm_j + 2)/2; Act scale=-2 restores d2.
    d_hi, d_lo = _fp8_hilo((SQC - qnorm) / 2.0)

    counts = np.bincount(Mx, minlength=1)
    n_sel = counts[Mx].astype(np.float32) - 1.0
    invn_full = np.where(n_sel > 0, 1.0 / np.maximum(n_sel, 1.0),
                         0.0).astype(np.float32)

    # padded-row lookups: index B maps to a zero row / label -1. The pad
    # "d" value is arbitrary (masked out) — 128.0 is fp8-exact (e4m3 max
    # finite is 240) and keeps every pad d2 strictly positive.
    zq_pad = np.vstack([zq, np.zeros((1, D), dtype=FP8)])
    d_hi = np.concatenate([d_hi, np.array([128.0], dtype=FP8)])
    d_lo = np.concatenate([d_lo, np.array([0.0], dtype=FP8)])
    lab_pad = np.concatenate([Mx, [-1]])
    invn_pad = np.concatenate([invn_full, [0.0]]).astype(np.float32)

    ones = np.ones((nch, P), dtype=FP8)
    t32 = np.full((nch, P), 32.0, dtype=FP8)
    tm16 = np.full((nch, P), -16.0, dtype=FP8)
    pidx = np.arange(P)

    in_maps = []
    for core in range(NCORES):
        idx = slots[core * nch:(core + 1) * nch]          # [nch, P]
        idx0 = np.where(idx < 0, B, idx)
        g = zq_pad[idx0]                                   # [nch, P, D] fp8
        if dr:
            # zt[p, s, t, c, r] = Q[idx[c, r], t*256 + s*128 + p]
            zt = np.ascontiguousarray(
                g.reshape(nch, P, KT // 2, 2, P).transpose(4, 3, 2, 0, 1))
        else:
            # zt[p, t, c, r] = Q[idx[c, r], t*128+p]
            zt = np.ascontiguousarray(
                g.reshape(nch, P, KT, P).transpose(3, 2, 0, 1))
        ah, al = d_hi[idx0], d_lo[idx0]                    # [nch, P]
        aug_l_t = np.ascontiguousarray(
            np.stack([ah, al, ones, ones, t32]))           # [AUGK, nch, P]
        aug_r_t = np.ascontiguousarray(
            np.stack([ones, ones, ah, al, tm16]))
        lab = lab_pad[idx0]                                # [nch, P]
        valid = idx >= 0
        msk = ((lab[:, :, None] == lab[:, None, :])
               & valid[:, :, None] & valid[:, None, :])
        msk[:, pidx, pidx] = False
        in_maps.append({
            "zt": zt,
            "aug_l": aug_l_t,
            "aug_r": aug_r_t,
            # device mask layout [P(i), nch, P(j)]
            "mask": np.ascontiguousarray(
                msk.astype(BF16).transpose(1, 0, 2)),
            "invn": np.ascontiguousarray(invn_pad[idx0]),
            "rep_tag": np.zeros((1, max(repeat, 1)), np.float32),
        })
    return in_maps


def _plan(z: np.ndarray, Mx: np.ndarray, repeat: int = 1, dr: bool = True):
    """Returns (nc, in_maps, slots). slots maps device output back to rows."""
    slots = _pack(np.asarray(Mx).astype(np.int64))
    nch = slots.shape[0] // NCORES
    key = ("nc", repeat, nch, dr)
    if key not in _cache:
        nc = _build_bass(repeat=repeat, nch=nch, dr=dr)
        nc.finalize()
        _cache[key] = nc
    return (_cache[key],
            _prepare_inputs(z, Mx, slots, repeat=repeat, dr=dr), slots)


def _assemble(results, slots) -> np.ndarray:
    nch = slots.shape[0] // NCORES
    out = np.empty(B, dtype=np.float32)
    for core, r in enumerate(results):
        vals = r["loss"].reshape(nch * P)
        idx = slots[core * nch:(core + 1) * nch].reshape(nch * P)
        sel = idx >= 0
        out[idx[sel]] = vals[sel]
    return out


def kernel(z: np.ndarray, Mx: np.ndarray, **run_kwargs) -> np.ndarray:
    nc, in_maps, slots = _plan(z, Mx, repeat=1)
    res = run_bass_kernel_spmd(nc, in_maps, core_ids=list(range(NCORES)),
                               **run_kwargs)
    _cache["last_results"] = res
    return _assemble(res.results, slots)
